# revision 31
# baseline (speedup 1.0000x reference)
"""Trainium2 Bass kernel for ModalitySpecificLocalSelfAttention (7x7 window).

Spatial-parallel over H across 8 cores (16-row stripe + 3-row halo each).
v4.1 design (49.9us; v4 52.9us; v3 61.5us). Pair-granular v epilogues
(adjacent PSUM banks, one op per 2 blocks); conv epilogues measured
cheaper as singles (DVE 2x mode) so they stay per-chunk.
  - Rolling PSUM bank tracker (regions [0:308] for v/S/conv, [384:512]
    for AV) replaces v3's hand-woven tenancy waits; emission order makes
    every exp/acopy pair land on adjacent banks.
  - Dense conv+v phase (v blocks interleaved with k1 chunks as xs DMA
    chunks land), then S quads with per-block exp + accum_out (z produced
    by the ACT pass; kills v3's 6.3us of DVE reduces), then AV pairs.
  - smask deduped to 6 variants (row-class x col-class) instead of 16
    block masks: input drops 790KB of DMA.
  - Weight DMA split: wallA (wk1t/wvt/wq1t) first on the sync ring so the
    first matmul starts ~0.7us earlier; wallB rides the scalar ring.
  - All transposes quad-batched on the sync ring; act-table load warmed
    by a dummy op at t=0 so its 1.3us hides under the input DMAs.
  CAUTION: removing the exp->norm slot-pacing waits or widening the
  softmax buffers to 16 slots hard-crashes the device (deterministic
  INTERNAL error; suspected cayman event-accel deadlock from unspaced
  multi-engine then_inc streams). Keep the 8-slot structure.
"""

import sys

for _p in ("/opt/trn_rl_repo", "/root/.axon_site/_ro/trn_rl_repo"):
    if _p not in sys.path:
        sys.path.append(_p)

import ml_dtypes
import numpy as np

import concourse.bass as bass
from concourse import mybir
from concourse.bass_utils import run_bass_kernel_spmd

F32 = mybir.dt.float32
BF16 = mybir.dt.bfloat16

C = 128
H = 128
W = 128
NCORES = 8
RPC = H // NCORES          # 16 rows per core
PAD = 3
HALO = RPC + 2 * PAD       # 22 rows incl halo
WP = W + 2 * PAD           # 134 padded width
BR, BC = 8, 16             # pixel block 8 rows x 16 cols
NR, NC_ = BR + 2 * PAD, BC + 2 * PAD   # 14 x 22 neighborhood
NN = NR * NC_              # 308
NN2 = 384                  # padded to 3x128 for the xbar transpose
NPIX = RPC * W             # 2048
NXP = HALO * WP            # 2948 padded stripe pixels
CH = 512
NVAR = 6                   # mask variants: rowclass(2) x colclass(3)
EXP_SHIFT = -16.0
MASKV = -40.0
DEBUG_OUTS = False

# xs input chunking (4 chunks over the flattened [C, NXP])
XCH = 4
XSPL = [0, 737, 1474, 2211, NXP]

# engine per softmax-normalize block (gp = GPSIMD is ~15ns/elem — useless;
# measured 4.6us per 308-elem norm)
NORM_ENG = ["dve"] * 16

RELU = mybir.ActivationFunctionType.Relu
IDENT = mybir.ActivationFunctionType.Identity
EXP = mybir.ActivationFunctionType.Exp
COPY = mybir.ActivationFunctionType.Copy
ADD = mybir.AluOpType.add
MULT = mybir.AluOpType.mult
MAXOP = mybir.AluOpType.max


def _vmap(b):
    """block index -> mask variant slot"""
    rc = b // 8
    cb = b % 8
    cc = 0 if cb == 0 else (2 if cb == 7 else 1)
    return rc * 3 + cc


def _build_program():
    nc = bass.Bass("TRN2", target_bir_lowering=False, debug=False)

    # ---- DRAM I/O ----
    xs_d = nc.dram_tensor("xs", [C, NXP], BF16, kind="ExternalInput").ap()
    wa_d = nc.dram_tensor("walla", [C, 3 * C], BF16, kind="ExternalInput").ap()
    wb_d = nc.dram_tensor("wallb", [C, 5 * C], BF16, kind="ExternalInput").ap()
    ball_d = nc.dram_tensor("ball", [C, 8], F32, kind="ExternalInput").ap()
    smask_d = nc.dram_tensor("smask", [C, NVAR * NN], BF16,
                             kind="ExternalInput").ap()
    oobc_d = nc.dram_tensor("oobc", [C, 16], F32, kind="ExternalInput").ap()
    y_d = nc.dram_tensor("y", [C, NPIX], BF16, kind="ExternalOutput").ap()

    # ---- SBUF ----
    sb = lambda name, shape, dt: nc.alloc_sbuf_tensor(name, list(shape), dt).ap()
    xsp = sb("xsp_sb", [C, HALO, WP], BF16)
    k1 = sb("k1_sb", [C, HALO * W], BF16)
    q1 = sb("q1_sb", [C, NPIX], BF16)
    q = sb("q_sb", [C, 16, C], BF16)       # block-major
    kpad = sb("kpad_sb", [C, HALO, WP], BF16)
    vn16 = sb("vn16_sb", [C, 16, NN2], BF16)
    vt16 = sb("vt16_sb", [C, 16, 3, C], BF16)
    e8 = sb("e8_sb", [C, 8, NN], BF16)
    am8 = sb("am8_sb", [C, 8, NN2], BF16)
    at8 = sb("at8_sb", [C, 8, 3, C], BF16)
    z16 = sb("z16_sb", [C, 16], F32)
    rz16 = sb("rz16_sb", [C, 16], F32)
    attn = sb("attn_sb", [C, RPC, W], BF16)
    walla = sb("walla_sb", [C, 3, C], BF16)
    wallb = sb("wallb_sb", [C, 5, C], BF16)
    ball = sb("ball_sb", [C, 8], F32)
    maskM = sb("maskM_sb", [C, NVAR, NN], BF16)
    oobc16 = sb("oobc16_sb", [C, 16], F32)
    eshift = sb("eshift_sb", [C, 1], F32)
    warm = sb("warm_sb", [C, 1], F32)
    yt = sb("yt_sb", [C, 2, CH], BF16)

    WA_IDX = {"wk1t": 0, "wvt": 1, "wq1t": 2}
    WB_IDX = {"wk2t": 0, "wq2t": 1, "wat": 2, "wxt": 3, "id": 4}
    w_sb = {n: walla[:, k, :] for n, k in WA_IDX.items()}
    w_sb.update({n: wallb[:, k, :] for n, k in WB_IDX.items()})
    B_IDX = {n: k for k, n in enumerate(
        ("bq1", "bq2", "bk1", "bk2", "bv", "bo"))}
    b_sb = {n: ball[:, k:k + 1] for n, k in B_IDX.items()}

    ps = nc.alloc_psum_tensor("ps", [C, 8, CH], F32).ap()

    # ---- semaphores / plan ----
    sem_names = ("sdwa", "sdwb", "sdball", "sdo", "sdm", "sp", "sa", "sv",
                 "sg", "sdvt", "sdat", "sdout") + tuple(
                     f"sdx{j}" for j in range(XCH))
    sems = {n: nc.alloc_semaphore(n) for n in sem_names}
    ENGS = ("sync", "pe", "act", "dve", "gp")
    plan = {e: [] for e in ENGS}
    cnt = {n: 0 for n in sem_names}

    def op(eng, fn, sem, inc=1):
        plan[eng].append(("op", fn, sem, inc))
        if sem:
            cnt[sem] += inc
            return (sem, cnt[sem])
        return None

    def wait(eng, mark):
        if mark is not None:
            sem, val = mark
            if val and val > 0:
                plan[eng].append(("w", sem, val))

    # ---- init: GP memsets, DVE eshift, ACT table warmup ----
    for c0 in (0, WP - PAD):
        op("gp", lambda tf=kpad[:, :, c0:c0 + PAD]: nc.gpsimd.memset(tf, 0.0),
           "sg")
    op("gp", lambda: nc.gpsimd.memset(vn16[:, :, NN:NN2], 0.0), "sg")
    op("gp", lambda: nc.gpsimd.memset(am8[:, :, NN:NN2], 0.0), "sg")
    MEMSETS = ("sg", cnt["sg"])
    ESHIFT = op("dve", lambda: nc.vector.memset(eshift, EXP_SHIFT), "sv")
    wait("act", ESHIFT)
    op("act", lambda: nc.scalar.activation(warm, eshift, RELU), "sa")

    # ---- input DMAs ----
    def dma(eng, sem, dst, srcd):
        fn = nc.sync.dma_start if eng == "sync" else nc.scalar.dma_start
        return op(eng, lambda d=dst, s=srcd, f=fn: f(out=d, in_=s), sem, 16)

    xsp_f = xsp.rearrange("p r w -> p (r w)")
    XS = [None] * XCH

    def xs_dma(eng, i):
        lo, hi = XSPL[i], XSPL[i + 1]
        XS[i] = dma(eng, f"sdx{i}", xsp_f[:, lo:hi], xs_d[:, lo:hi])

    WALLA = dma("sync", "sdwa", walla.rearrange("p a b -> p (a b)"), wa_d)
    xs_dma("sync", 0)
    xs_dma("act", 1)
    WALLB = dma("act", "sdwb", wallb.rearrange("p a b -> p (a b)"), wb_d)
    xs_dma("sync", 2)
    xs_dma("sync", 3)
    BALL = dma("act", "sdball", ball, ball_d)
    OOBC = dma("act", "sdo", oobc16, oobc_d)
    SMASK = dma("sync", "sdm", maskM.rearrange("p a b -> p (a b)"), smask_d)

    def xdeps(eng, row0, row1):
        c0 = 0
        c1 = XCH - 1
        for c in range(XCH):
            if XSPL[c + 1] > row0 * WP:
                c0 = c
                break
        for c in range(XCH):
            if XSPL[c + 1] >= row1 * WP:
                c1 = c
                break
        for c in range(c0, c1 + 1):
            wait(eng, XS[c])

    mark = {}

    # ---- PSUM tenancy tracker ----
    bankA = [None] * 8       # region [0:308] freeing mark
    bankB = [None] * 8       # region [384:512] freeing mark
    seqA = [0]

    def claim(eng, full, bank=None):
        if bank is None:
            bank = seqA[0] % 8
            seqA[0] += 1
        wait(eng, bankA[bank])
        if full:
            wait(eng, bankB[bank])
        return bank

    epi_rr = [0]

    def next_epi_eng():
        epi_rr[0] ^= 1
        return "dve" if epi_rr[0] else "act"

    def emit_epi(eng, dst, src, bias, relu=True):
        if eng == "act":
            return op("act",
                      lambda o=dst, i_=src, b_=bias, f=(RELU if relu else IDENT):
                          nc.scalar.activation(o, i_, f, bias=b_),
                      "sa")
        return op("dve",
                  lambda o=dst, i_=src, b_=bias,
                         op1=(MAXOP if relu else mybir.AluOpType.bypass):
                      nc.vector.tensor_scalar(o, i_, b_, 0.0, ADD, op1),
                  "sv")

    # ---- v blocks ----
    def blk_geom(b):
        return 8 * (b // 8), (b % 8) * 16

    vbank = {}

    def emit_v(b):
        r0, c0 = blk_geom(b)
        bank = claim("pe", False)
        vbank[b] = bank
        xdeps("pe", r0, r0 + NR)
        mark[("vmm", b)] = op(
            "pe",
            lambda o=ps[:, bank, 0:NN], l=w_sb["wvt"],
                   r=xsp[:, r0:r0 + NR, c0:c0 + NC_]:
                nc.tensor.matmul(o, l, r, start=True, stop=True),
            "sp")

    def emit_vepi(b0):
        # pair epilogue: blocks b0, b0+1 land on adjacent banks by
        # construction of the claim order
        eng = next_epi_eng()
        bank = vbank[b0]
        assert vbank[b0 + 1] == bank + 1
        wait(eng, BALL)
        wait(eng, mark[("vmm", b0 + 1)])
        m = emit_epi(eng, vn16[:, b0:b0 + 2, 0:NN],
                     ps[:, bank:bank + 2, 0:NN], b_sb["bv"])
        mark[("vepi", b0)] = m
        mark[("vepi", b0 + 1)] = m
        bankA[bank] = m
        bankA[bank + 1] = m

    def emit_vt(g):
        wait("sync", MEMSETS)
        for b in range(4 * g, 4 * g + 4):
            wait("sync", mark[("vepi", b)])
        mark[("vt", g)] = op(
            "sync",
            lambda o=vt16[:, 4 * g:4 * g + 4], i_=vn16[:, 4 * g:4 * g + 4, :]:
                nc.sync.dma_start(out=o, in_=i_, transpose=True),
            "sdvt", 16)

    # ---- conv chunks ----
    def chunk_cols(cname, j):
        tot = HALO * W if cname in ("k1", "k2") else NPIX
        return min(CH, tot - j * CH)

    CONV_W = {"k1": "wk1t", "q1": "wq1t", "k2": "wk2t", "q2": "wq2t"}
    CONV_B = {"k1": "bk1", "q1": "bq1", "k2": "bk2", "q2": "bq2"}
    cbank = {}

    def emit_conv(cname, j):
        n = chunk_cols(cname, j)
        nrows = n // W
        bank = claim("pe", True)
        cbank[(cname, j)] = bank
        if cname == "k1":
            if j == 0:
                wait("pe", WALLA)
            xdeps("pe", 4 * j, 4 * j + nrows)
            rhs = xsp[:, 4 * j:4 * j + nrows, PAD:PAD + W]
        elif cname == "q1":
            xdeps("pe", PAD + 4 * j, PAD + 4 * j + nrows)
            rhs = xsp[:, PAD + 4 * j:PAD + 4 * j + nrows, PAD:PAD + W]
        elif cname == "k2":
            if j == 0:
                wait("pe", WALLB)
            wait("pe", mark[("cepi", ("k1", j))])
            rhs = k1[:, bass.ds(j * CH, n)]
        else:
            wait("pe", mark[("cepi", ("q1", j))])
            rhs = q1[:, bass.ds(j * CH, n)]
        mark[("cmm", (cname, j))] = op(
            "pe",
            lambda o=ps[:, bank, :n], l=w_sb[CONV_W[cname]], r=rhs:
                nc.tensor.matmul(o, l, r, start=True, stop=True),
            "sp")

    def emit_cepi(cname, j):
        eng = next_epi_eng()
        bank = cbank[(cname, j)]
        n = chunk_cols(cname, j)
        nrows = n // W
        wait(eng, BALL)
        wait(eng, mark[("cmm", (cname, j))])
        src = ps[:, bank, :n]
        if cname == "k1":
            dst = k1[:, bass.ds(j * CH, n)]
        elif cname == "q1":
            dst = q1[:, bass.ds(j * CH, n)]
        elif cname == "k2":
            dst = kpad[:, 4 * j:4 * j + nrows, PAD:PAD + W]
            src = src.rearrange("p (r w) -> p r w", w=W)
        else:
            br = j // 2
            rlo = (j % 2) * 4
            dst = q[:, br * 8:br * 8 + 8,
                    rlo * BC:(rlo + 4) * BC].rearrange(
                        "p b (r w) -> p b r w", w=BC)
            src = src.rearrange("p (r b w) -> p b r w", r=4, w=BC)
        m = emit_epi(eng, dst, src, b_sb[CONV_B[cname]])
        mark[("cepi", (cname, j))] = m
        bankA[bank] = m
        bankB[bank] = m

    def emit_cepi_pair(cname, j0):
        # chunks j0, j0+1 on adjacent banks (claim-order invariant)
        eng = next_epi_eng()
        bank = cbank[(cname, j0)]
        assert cbank[(cname, j0 + 1)] == bank + 1
        assert chunk_cols(cname, j0) == CH and chunk_cols(cname, j0 + 1) == CH
        wait(eng, BALL)
        wait(eng, mark[("cmm", (cname, j0 + 1))])
        src = ps[:, bank:bank + 2, :].rearrange("p a b -> p (a b)")
        if cname == "k1":
            dst = k1[:, bass.ds(j0 * CH, 2 * CH)]
        elif cname == "q1":
            dst = q1[:, bass.ds(j0 * CH, 2 * CH)]
        elif cname == "k2":
            dst = kpad[:, 4 * j0:4 * j0 + 8, PAD:PAD + W]
            src = src.rearrange("p (r w) -> p r w", w=W)
        else:   # q2 pair covers all 8 rows of one block-row
            br = j0 // 2
            dst = q[:, br * 8:br * 8 + 8, :].rearrange(
                "p b (r w) -> p r b w", w=BC)
            src = src.rearrange("p (r b w) -> p r b w", r=8, w=BC)
        m = emit_epi(eng, dst, src, b_sb[CONV_B[cname]])
        for j in (j0, j0 + 1):
            mark[("cepi", (cname, j))] = m
        bankA[bank] = m
        bankA[bank + 1] = m
        bankB[bank] = m
        bankB[bank + 1] = m

    # ---- attention ----
    sbank = {}

    def emit_S(b):
        r0, c0 = blk_geom(b)
        bank = claim("pe", False)
        sbank[b] = bank
        for j in range(r0 // 4, (r0 + NR + 3) // 4):
            if j < 6:
                wait("pe", mark[("cepi", ("k2", j))])
        for j in ((0, 1) if b < 8 else (2, 3)):
            wait("pe", mark[("cepi", ("q2", j))])
        if b == 0:
            wait("pe", SMASK)
        mark[("smm", b)] = op(
            "pe",
            lambda o=ps[:, bank, 0:NN], l=q[:, b, :],
                   r=kpad[:, r0:r0 + NR, c0:c0 + NC_]:
                nc.tensor.matmul(o, l, r, start=True, stop=False),
            "sp")
        mark[("mmm", b)] = op(
            "pe",
            lambda o=ps[:, bank, 0:NN], l=w_sb["id"], r=maskM[:, _vmap(b), :]:
                nc.tensor.matmul(o, l, r, start=False, stop=True),
            "sp")

    def emit_exp_pair(b0):
        s0 = b0 % 8
        bank = sbank[b0]
        assert sbank[b0 + 1] == bank + 1
        if b0 == 0:
            wait("act", ESHIFT)
        wait("act", mark[("mmm", b0 + 1)])
        if b0 >= 8:
            wait("act", mark[("norm", b0 - 8)])
            wait("act", mark[("norm", b0 - 7)])
        m = op("act",
               lambda o=e8[:, s0:s0 + 2, :], i_=ps[:, bank:bank + 2, 0:NN],
                      sh=eshift:
                   nc.scalar.activation(o, i_, EXP, bias=sh),
               "sa")
        mark[("exp", b0)] = m
        bankA[bank] = m
        bankA[bank + 1] = m

    def emit_zchain(b0):   # reduce + oobc add + recip, pair, DVE
        s0 = b0 % 8
        wait("dve", mark[("exp", b0)])
        if b0 == 0:
            wait("dve", OOBC)
        zm = op("dve",
                lambda o=z16[:, b0:b0 + 2], i_=e8[:, s0:s0 + 2, :]:
                    nc.vector.reduce_sum(o, i_, axis=mybir.AxisListType.X),
                "sv")
        wait("dve", zm)
        zm = op("dve",
                lambda o=z16[:, b0:b0 + 2], i_=z16[:, b0:b0 + 2],
                       i1=oobc16[:, b0:b0 + 2]:
                    nc.vector.tensor_add(o, i_, i1),
                "sv")
        wait("dve", zm)
        rm = op("dve",
                lambda o=rz16[:, b0:b0 + 2], i_=z16[:, b0:b0 + 2]:
                    nc.vector.reciprocal(o, i_),
                "sv")
        mark[("rz", b0)] = rm

    def emit_norm(b):
        s = b % 8
        g = b // 4
        eng = NORM_ENG[b]
        wait(eng, mark[("rz", b - b % 2)])
        if g >= 2:
            wait(eng, mark[("at", g - 2)])
        if eng == "gp":
            wait(eng, MEMSETS)   # keep gp stream ordered anyway
            mark[("norm", b)] = op(
                "gp",
                lambda o=am8[:, s, 0:NN], i_=e8[:, s, :], sc=rz16[:, b:b + 1]:
                    nc.gpsimd.tensor_scalar_mul(o, i_, sc),
                "sg")
        else:
            mark[("norm", b)] = op(
                "dve",
                lambda o=am8[:, s, 0:NN], i_=e8[:, s, :], sc=rz16[:, b:b + 1]:
                    nc.vector.tensor_scalar_mul(o, i_, sc),
                "sv")

    def emit_at(g):
        s0 = (4 * g) % 8
        for b in range(4 * g, 4 * g + 4):
            wait("sync", mark[("norm", b)])
        if g == 0:
            wait("sync", MEMSETS)
        if g >= 2:
            wait("sync", mark[("avmm", 4 * (g - 2) + 3)])
        mark[("at", g)] = op(
            "sync",
            lambda o=at8[:, s0:s0 + 4], i_=am8[:, s0:s0 + 4, :]:
                nc.sync.dma_start(out=o, in_=i_, transpose=True),
            "sdat", 16)

    def emit_av(b):
        s = b % 8
        g = b // 4
        bank = sbank[b]
        wait("pe", bankB[bank])
        wait("pe", mark[("vt", g)])
        wait("pe", mark[("at", g)])
        for ch in range(3):
            mark[("avmm", b)] = op(
                "pe",
                lambda o=ps[:, bank, NN2:CH], l=vt16[:, b, ch, :],
                       r=at8[:, s, ch, :], st=(ch == 0), sp_=(ch == 2):
                    nc.tensor.matmul(o, l, r, start=st, stop=sp_),
                "sp")

    def emit_acopy(b):   # pair: blocks b, b+1 on adjacent banks
        r0, c0 = blk_geom(b)
        bank = sbank[b]
        assert sbank[b + 1] == bank + 1
        eng = next_epi_eng()
        wait(eng, mark[("avmm", b + 1)])
        dst = attn[:, r0:r0 + BR, c0:c0 + 2 * BC].rearrange(
            "p r (a w) -> p a r w", w=BC)
        src = ps[:, bank:bank + 2, NN2:CH].rearrange(
            "p a (r w) -> p a r w", w=BC)
        if eng == "act":
            m = op("act", lambda o=dst, i_=src: nc.scalar.copy(o, i_), "sa")
        else:
            m = op("dve",
                   lambda o=dst, i_=src: nc.vector.tensor_scalar(
                       o, i_, 0.0, None, ADD),
                   "sv")
        mark[("acopy", b)] = m
        bankB[bank] = m
        bankB[bank + 1] = m

    def emit_avpair(b0):
        emit_av(b0)
        emit_av(b0 + 1)
        emit_acopy(b0)

    # ---- output conv: column chunks, rolling banks ----
    def emit_o(i):
        bank = claim("pe", True)
        wait("pe", mark[("acopy", 2 * i)])
        wait("pe", mark[("acopy", 8 + 2 * i)])
        op("pe",
           lambda o=ps[:, bank, :], l=w_sb["wat"],
                  r=attn[:, :, 32 * i:32 * i + 32]:
               nc.tensor.matmul(o, l, r, start=True, stop=False),
           "sp")
        om = op("pe",
                lambda o=ps[:, bank, :], l=w_sb["wxt"],
                       r=xsp[:, PAD:PAD + RPC, PAD + 32 * i:PAD + 32 * i + 32]:
                    nc.tensor.matmul(o, l, r, start=False, stop=True),
                "sp")
        eng = next_epi_eng()
        wait(eng, om)
        if i >= 2:
            wait(eng, mark[("odma", i - 2)])
        m = emit_epi(eng, yt[:, i % 2, :], ps[:, bank, :], b_sb["bo"],
                     relu=False)
        mark[("oepi", i)] = m
        bankA[bank] = m
        bankB[bank] = m
        wait("sync", m)
        mark[("odma", i)] = op(
            "sync",
            lambda o=y_d[:, bass.ts(i, CH)], i_=yt[:, i % 2, :]:
                nc.sync.dma_start(out=o, in_=i_),
            "sdout", 16)

    # =====================================================================
    # Emission schedule
    def emit_S_quad(g):
        for b in range(4 * g, 4 * g + 4):
            emit_S(b)
            if b % 2:
                emit_exp_pair(b - 1)
        for p in (0, 2):
            b0 = 4 * g + p
            emit_zchain(b0)
            emit_norm(b0)
            emit_norm(b0 + 1)

    # v4.0-style skeleton: dense conv+v phase, then S quads, then AV pairs.
    emit_conv("k1", 0)
    for b in range(4):
        emit_v(b)
        if b % 2:
            emit_vepi(b - 1)
    emit_conv("k1", 1)
    emit_cepi("k1", 0)
    for b in range(4, 8):
        emit_v(b)
        if b % 2:
            emit_vepi(b - 1)
    emit_vt(0)
    emit_conv("k1", 2)
    emit_cepi("k1", 1)
    emit_conv("k1", 3)
    emit_cepi("k1", 2)
    for b in range(8, 12):
        emit_v(b)
        if b % 2:
            emit_vepi(b - 1)
    emit_vt(1)
    emit_conv("k1", 4)
    emit_cepi("k1", 3)
    emit_conv("k1", 5)
    emit_cepi("k1", 4)
    for b in range(12, 16):
        emit_v(b)
        if b % 2:
            emit_vepi(b - 1)
    emit_vt(2)
    emit_conv("q1", 0)
    emit_cepi("k1", 5)
    emit_cepi("q1", 0)
    for j in range(1, 4):
        emit_conv("q1", j)
        emit_cepi("q1", j)
    emit_vt(3)
    emit_conv("k2", 4)
    emit_cepi("k2", 4)
    emit_conv("k2", 5)
    emit_cepi("k2", 5)
    emit_conv("k2", 0)
    emit_cepi("k2", 0)
    emit_conv("k2", 1)
    emit_cepi("k2", 1)
    emit_conv("k2", 2)
    emit_cepi("k2", 2)
    emit_conv("k2", 3)
    emit_cepi("k2", 3)
    emit_conv("q2", 0)
    emit_cepi("q2", 0)
    emit_conv("q2", 1)
    emit_cepi("q2", 1)
    emit_S_quad(0)
    emit_at(0)
    emit_conv("q2", 2)
    emit_cepi("q2", 2)
    emit_conv("q2", 3)
    emit_cepi("q2", 3)
    emit_S_quad(1)
    emit_at(1)
    emit_S_quad(2)
    emit_S_quad(3)
    emit_avpair(0)
    emit_avpair(2)
    emit_at(2)
    emit_avpair(4)
    emit_avpair(6)
    emit_at(3)
    emit_avpair(8)
    emit_avpair(10)
    emit_o(0)
    emit_o(1)
    emit_avpair(12)
    emit_avpair(14)
    emit_o(2)
    emit_o(3)

    if DEBUG_OUTS:
        dbg = {
            "d_q": q.rearrange("p a b -> p (a b)"),
            "d_kpad": kpad.rearrange("p r w -> p (r w)"),
            "d_vn": vn16.rearrange("p a b -> p (a b)"),
            "d_vt": vt16.rearrange("p a b c -> p (a b c)"),
            "d_attn": attn.rearrange("p r w -> p (r w)"),
            "d_z": z16,
            "d_rz": rz16,
            "d_am": am8.rearrange("p a b -> p (a b)"),
            "d_at": at8.rearrange("p a b c -> p (a b c)"),
        }
        for nm, src in dbg.items():
            dd = nc.dram_tensor(nm, list(src.shape),
                                src.dtype, kind="ExternalOutput").ap()
            for s_ in ("sp", "sa", "sv", "sg"):
                wait("sync", (s_, cnt[s_]))
            op("sync", lambda o=dd, i_=src: nc.sync.dma_start(out=o, in_=i_),
               "sdout", 16)

    # ---- tail barrier ----
    for s_ in ("sp", "sa", "sv", "sg", "sdvt", "sdat", "sdout",
               "sdwa", "sdwb", "sdball", "sdo", "sdm"):
        wait("sync", (s_, cnt[s_]))
    for j in range(XCH):
        wait("sync", (f"sdx{j}", cnt[f"sdx{j}"]))

    # ---- emit ----
    def run(eng_name, eng_obj):
        hwm = {}
        for item in plan[eng_name]:
            if item[0] == "w":
                _, s_, v = item
                if hwm.get(s_, 0) >= v:
                    continue
                hwm[s_] = v
                eng_obj.wait_ge(sems[s_], v)
            else:
                _, fn, s_, inc = item
                inst = fn()
                if s_:
                    inst.then_inc(sems[s_], inc)

    with nc.Block() as block:
        @block.sync
        def _(e):
            run("sync", e)

        @block.tensor
        def _(e):
            run("pe", e)

        @block.scalar
        def _(e):
            run("act", e)

        @block.vector
        def _(e):
            run("dve", e)

        @block.gpsimd
        def _(e):
            run("gp", e)

    return nc


_PROGRAM = None


def _host_inputs(x, w_q1, s_q1, b_q1, w_q2, s_q2, b_q2,
                 w_k1, s_k1, b_k1, w_k2, s_k2, b_k2,
                 w_v, s_v, b_v, w_o, s_o, b_o):
    def foldT(w, s):
        return np.ascontiguousarray((s[:, None] * w).T.astype(ml_dtypes.bfloat16))

    wq1t, wq2t = foldT(w_q1, s_q1), foldT(w_q2, s_q2)
    wk1t, wk2t = foldT(w_k1, s_k1), foldT(w_k2, s_k2)
    wvt = foldT(w_v, s_v)
    wo = s_o[:, None] * w_o
    wat = np.ascontiguousarray(wo[:, :C].T.astype(ml_dtypes.bfloat16))
    wxt = np.ascontiguousarray(wo[:, C:].T.astype(ml_dtypes.bfloat16))

    col = lambda b: np.ascontiguousarray(b.astype(np.float32)[:, None])

    valid = np.zeros((BR * BC, NR, NC_), bool)
    for r in range(BR):
        for c in range(BC):
            p = r * BC + c
            valid[p, r:r + 7, c:c + 7] = True

    X = np.asarray(x, np.float32).reshape(C, H, W)
    wallA = np.concatenate([wk1t, wvt, wq1t], axis=1)
    wallB = np.concatenate(
        [wk2t, wq2t, wat, wxt, np.eye(C, dtype=ml_dtypes.bfloat16)], axis=1)
    shared = dict(walla=np.ascontiguousarray(wallA),
                  wallb=np.ascontiguousarray(wallB))

    e16v = np.float32(np.exp(EXP_SHIFT))
    var_rep = {rc * 3 + cc: (rc, {0: 0, 1: 3, 2: 7}[cc])
               for rc in range(2) for cc in range(3)}

    in_maps = []
    for core in range(NCORES):
        h0 = core * RPC
        xsb = np.zeros((C, HALO, WP), np.float32)
        lo, hi = h0 - PAD, h0 + RPC + PAD
        slo, shi = max(lo, 0), min(hi, H)
        xsb[:, slo - lo:shi - lo, PAD:PAD + W] = X[:, slo:shi]

        maskm = np.empty((NVAR, BR * BC, NN), np.float32)
        for v, (brr, cb) in var_rep.items():
            rowok = np.array([0 <= h0 + brr * BR + ri - PAD < H
                              for ri in range(NR)])
            colok = np.array([0 <= cb * BC + ci - PAD < W
                              for ci in range(NC_)])
            inimg = rowok[:, None] & colok[None, :]
            mb = np.where(valid & inimg[None, :, :], 0.0, MASKV)
            maskm[v] = mb.reshape(BR * BC, NN)
        oobc = np.empty((16, BR * BC), np.float32)
        for b in range(16):
            brr, cb = b // 8, b % 8
            rowok = np.array([0 <= h0 + brr * BR + ri - PAD < H
                              for ri in range(NR)])
            colok = np.array([0 <= cb * BC + ci - PAD < W
                              for ci in range(NC_)])
            inimg = rowok[:, None] & colok[None, :]
            n_oob = (valid & ~inimg[None, :, :]).sum(axis=(1, 2))
            oobc[b] = n_oob * e16v
        m = dict(shared)
        m["xs"] = np.ascontiguousarray(
            xsb.reshape(C, NXP).astype(ml_dtypes.bfloat16))
        m["smask"] = np.ascontiguousarray(
            maskm.transpose(1, 0, 2).reshape(BR * BC, NVAR * NN)
            .astype(ml_dtypes.bfloat16))
        m["oobc"] = np.ascontiguousarray(oobc.T.astype(np.float32))
        m["ball"] = np.ascontiguousarray(np.concatenate(
            [col(b_q1), col(b_q2), col(b_k1), col(b_k2), col(b_v),
             col(b_o), np.zeros((C, 2), np.float32)], axis=1))
        in_maps.append(m)
    return in_maps


def kernel(**inputs):
    global _PROGRAM
    if _PROGRAM is None:
        _PROGRAM = _build_program()
    in_maps = _host_inputs(**{k: np.asarray(v) for k, v in inputs.items()})
    res = run_bass_kernel_spmd(_PROGRAM, in_maps, core_ids=list(range(NCORES)))
    stripes = [np.asarray(r["y"]).astype(np.float32)
               .reshape(C, 4, RPC, 32).transpose(0, 2, 1, 3).reshape(C, RPC, W)
               for r in res.results]
    return np.concatenate(stripes, axis=1).reshape(1, C, H, W)


if __name__ == "__main__":
    rng = np.random.default_rng(0)
    fake = {"x": rng.standard_normal((1, C, H, W)).astype(np.float32)}
    for n in ("q1", "q2", "k1", "k2", "v", "o"):
        cin = 2 * C if n == "o" else C
        fake["w_" + n] = (rng.standard_normal((C, cin)) / np.sqrt(cin)).astype(np.float32)
        fake["s_" + n] = rng.uniform(0.5, 1.5, C).astype(np.float32)
        fake["b_" + n] = (rng.standard_normal(C) * 0.1).astype(np.float32)
    out = kernel(**fake)
    print("kernel output", out.shape, out.dtype)


# revision 32
# speedup vs baseline: 1.2561x; 1.2561x over previous
"""Trainium2 Bass kernel for ModalitySpecificLocalSelfAttention (7x7 window).

Spatial-parallel over H across 8 cores (16-row stripe + 3-row halo each).
v4.1 design (49.9us; v4 52.9us; v3 61.5us). Pair-granular v epilogues
(adjacent PSUM banks, one op per 2 blocks); conv epilogues measured
cheaper as singles (DVE 2x mode) so they stay per-chunk.
  - Rolling PSUM bank tracker (regions [0:308] for v/S/conv, [384:512]
    for AV) replaces v3's hand-woven tenancy waits; emission order makes
    every exp/acopy pair land on adjacent banks.
  - Dense conv+v phase (v blocks interleaved with k1 chunks as xs DMA
    chunks land), then S quads with per-block exp + accum_out (z produced
    by the ACT pass; kills v3's 6.3us of DVE reduces), then AV pairs.
  - smask deduped to 6 variants (row-class x col-class) instead of 16
    block masks: input drops 790KB of DMA.
  - Weight DMA split: wallA (wk1t/wvt/wq1t) first on the sync ring so the
    first matmul starts ~0.7us earlier; wallB rides the scalar ring.
  - All transposes quad-batched on the sync ring; act-table load warmed
    by a dummy op at t=0 so its 1.3us hides under the input DMAs.
  CAUTION: removing the exp->norm slot-pacing waits or widening the
  softmax buffers to 16 slots hard-crashes the device (deterministic
  INTERNAL error; suspected cayman event-accel deadlock from unspaced
  multi-engine then_inc streams). Keep the 8-slot structure.
"""

import sys

for _p in ("/opt/trn_rl_repo", "/root/.axon_site/_ro/trn_rl_repo"):
    if _p not in sys.path:
        sys.path.append(_p)

import ml_dtypes
import numpy as np

import concourse.bass as bass
from concourse import mybir
from concourse.bass_utils import run_bass_kernel_spmd

F32 = mybir.dt.float32
BF16 = mybir.dt.bfloat16

C = 128
H = 128
W = 128
NCORES = 8
RPC = H // NCORES          # 16 rows per core
PAD = 3
HALO = RPC + 2 * PAD       # 22 rows incl halo
WP = W + 2 * PAD           # 134 padded width
BR, BC = 8, 16             # pixel block 8 rows x 16 cols
NR, NC_ = BR + 2 * PAD, BC + 2 * PAD   # 14 x 22 neighborhood
NN = NR * NC_              # 308
NN2 = 384                  # padded to 3x128 for the xbar transpose
NPIX = RPC * W             # 2048
NXP = HALO * WP            # 2948 padded stripe pixels
CH = 512
NVAR = 6                   # mask variants: rowclass(2) x colclass(3)
EXP_SHIFT = -16.0
MASKV = -40.0
DEBUG_OUTS = False

# xs input chunking (4 chunks over the flattened [C, NXP])
XCH = 4
XSPL = [0, 737, 1474, 2211, NXP]

# engine per softmax-normalize block (gp = GPSIMD is ~15ns/elem — useless;
# measured 4.6us per 308-elem norm)
NORM_ENG = ["dve"] * 16

RELU = mybir.ActivationFunctionType.Relu
IDENT = mybir.ActivationFunctionType.Identity
EXP = mybir.ActivationFunctionType.Exp
COPY = mybir.ActivationFunctionType.Copy
ADD = mybir.AluOpType.add
MULT = mybir.AluOpType.mult
MAXOP = mybir.AluOpType.max


def _vmap(b):
    """block index -> mask variant slot"""
    rc = b // 8
    cb = b % 8
    cc = 0 if cb == 0 else (2 if cb == 7 else 1)
    return rc * 3 + cc


def _build_program():
    nc = bass.Bass("TRN2", target_bir_lowering=False, debug=False)

    # ---- DRAM I/O ----
    xs_d = nc.dram_tensor("xs", [C, NXP], BF16, kind="ExternalInput").ap()
    wa_d = nc.dram_tensor("walla", [C, 3 * C], BF16, kind="ExternalInput").ap()
    wb_d = nc.dram_tensor("wallb", [C, 5 * C], BF16, kind="ExternalInput").ap()
    ball_d = nc.dram_tensor("ball", [C, 8], F32, kind="ExternalInput").ap()
    smask_d = nc.dram_tensor("smask", [C, NVAR * NN], BF16,
                             kind="ExternalInput").ap()
    oobc_d = nc.dram_tensor("oobc", [C, 16], F32, kind="ExternalInput").ap()
    y_d = nc.dram_tensor("y", [C, NPIX], BF16, kind="ExternalOutput").ap()

    # ---- SBUF ----
    sb = lambda name, shape, dt: nc.alloc_sbuf_tensor(name, list(shape), dt).ap()
    xsp = sb("xsp_sb", [C, HALO, WP], BF16)
    k1 = sb("k1_sb", [C, HALO * W], BF16)
    q1 = sb("q1_sb", [C, NPIX], BF16)
    q = sb("q_sb", [C, 16, C], BF16)       # block-major
    kpad = sb("kpad_sb", [C, HALO, WP], BF16)
    vn16 = sb("vn16_sb", [C, 16, NN2], BF16)
    vt16 = sb("vt16_sb", [C, 16, 3, C], BF16)
    e8 = sb("e8_sb", [C, 8, NN], BF16)
    am8 = sb("am8_sb", [C, 8, NN2], BF16)
    at8 = sb("at8_sb", [C, 8, 3, C], BF16)
    z16 = sb("z16_sb", [C, 16], F32)
    rz16 = sb("rz16_sb", [C, 16], F32)
    attn = sb("attn_sb", [C, RPC, W], BF16)
    walla = sb("walla_sb", [C, 3, C], BF16)
    wallb = sb("wallb_sb", [C, 5, C], BF16)
    ball = sb("ball_sb", [C, 8], F32)
    maskM = sb("maskM_sb", [C, NVAR, NN], BF16)
    oobc16 = sb("oobc16_sb", [C, 16], F32)
    eshift = sb("eshift_sb", [C, 1], F32)
    warm = sb("warm_sb", [C, 1], F32)
    yt = sb("yt_sb", [C, 2, CH], BF16)

    WA_IDX = {"wk1t": 0, "wvt": 1, "wq1t": 2}
    WB_IDX = {"wk2t": 0, "wq2t": 1, "wat": 2, "wxt": 3, "id": 4}
    w_sb = {n: walla[:, k, :] for n, k in WA_IDX.items()}
    w_sb.update({n: wallb[:, k, :] for n, k in WB_IDX.items()})
    B_IDX = {n: k for k, n in enumerate(
        ("bq1", "bq2", "bk1", "bk2", "bv", "bo"))}
    b_sb = {n: ball[:, k:k + 1] for n, k in B_IDX.items()}

    ps = nc.alloc_psum_tensor("ps", [C, 8, CH], F32).ap()

    # ---- semaphores / plan ----
    sem_names = ("sdwa", "sdwb", "sdball", "sdo", "sdm", "sp", "sa", "sv",
                 "sg", "sdvt", "sdat", "sdout") + tuple(
                     f"sdx{j}" for j in range(XCH))
    sems = {n: nc.alloc_semaphore(n) for n in sem_names}
    ENGS = ("sync", "pe", "act", "dve", "gp")
    plan = {e: [] for e in ENGS}
    cnt = {n: 0 for n in sem_names}

    def op(eng, fn, sem, inc=1):
        plan[eng].append(("op", fn, sem, inc))
        if sem:
            cnt[sem] += inc
            return (sem, cnt[sem])
        return None

    def wait(eng, mark):
        if mark is not None:
            sem, val = mark
            if val and val > 0:
                plan[eng].append(("w", sem, val))

    # ---- init: GP memsets, DVE eshift, ACT table warmup ----
    for c0 in (0, WP - PAD):
        op("gp", lambda tf=kpad[:, :, c0:c0 + PAD]: nc.gpsimd.memset(tf, 0.0),
           "sg")
    op("gp", lambda: nc.gpsimd.memset(vn16[:, :, NN:NN2], 0.0), "sg")
    op("gp", lambda: nc.gpsimd.memset(am8[:, :, NN:NN2], 0.0), "sg")
    MEMSETS = ("sg", cnt["sg"])
    ESHIFT = op("dve", lambda: nc.vector.memset(eshift, EXP_SHIFT), "sv")
    wait("act", ESHIFT)
    op("act", lambda: nc.scalar.activation(warm, eshift, RELU), "sa")

    # ---- input DMAs ----
    def dma(eng, sem, dst, srcd):
        fn = nc.sync.dma_start if eng == "sync" else nc.scalar.dma_start
        return op(eng, lambda d=dst, s=srcd, f=fn: f(out=d, in_=s), sem, 16)

    xsp_f = xsp.rearrange("p r w -> p (r w)")
    XS = [None] * XCH

    def xs_dma(eng, i):
        lo, hi = XSPL[i], XSPL[i + 1]
        XS[i] = dma(eng, f"sdx{i}", xsp_f[:, lo:hi], xs_d[:, lo:hi])

    WALLA = dma("sync", "sdwa", walla.rearrange("p a b -> p (a b)"), wa_d)
    xs_dma("sync", 0)
    xs_dma("act", 1)
    WALLB = dma("act", "sdwb", wallb.rearrange("p a b -> p (a b)"), wb_d)
    xs_dma("sync", 2)
    xs_dma("sync", 3)
    BALL = dma("act", "sdball", ball, ball_d)
    OOBC = dma("act", "sdo", oobc16, oobc_d)
    SMASK = dma("sync", "sdm", maskM.rearrange("p a b -> p (a b)"), smask_d)

    def xdeps(eng, row0, row1):
        c0 = 0
        c1 = XCH - 1
        for c in range(XCH):
            if XSPL[c + 1] > row0 * WP:
                c0 = c
                break
        for c in range(XCH):
            if XSPL[c + 1] >= row1 * WP:
                c1 = c
                break
        for c in range(c0, c1 + 1):
            wait(eng, XS[c])

    mark = {}

    # ---- PSUM tenancy tracker ----
    bankA = [None] * 8       # region [0:308] freeing mark
    bankB = [None] * 8       # region [384:512] freeing mark
    seqA = [0]

    def claim(eng, full, bank=None):
        if bank is None:
            bank = seqA[0] % 8
            seqA[0] += 1
        wait(eng, bankA[bank])
        if full:
            wait(eng, bankB[bank])
        return bank

    epi_rr = [0]

    def next_epi_eng():
        epi_rr[0] ^= 1
        return "dve" if epi_rr[0] else "act"

    def emit_epi(eng, dst, src, bias, relu=True):
        if eng == "act":
            return op("act",
                      lambda o=dst, i_=src, b_=bias, f=(RELU if relu else IDENT):
                          nc.scalar.activation(o, i_, f, bias=b_),
                      "sa")
        return op("dve",
                  lambda o=dst, i_=src, b_=bias,
                         op1=(MAXOP if relu else mybir.AluOpType.bypass):
                      nc.vector.tensor_scalar(o, i_, b_, 0.0, ADD, op1),
                  "sv")

    # ---- v blocks ----
    def blk_geom(b):
        return 8 * (b // 8), (b % 8) * 16

    vbank = {}

    def emit_v(b):
        r0, c0 = blk_geom(b)
        bank = claim("pe", False)
        vbank[b] = bank
        xdeps("pe", r0, r0 + NR)
        mark[("vmm", b)] = op(
            "pe",
            lambda o=ps[:, bank, 0:NN], l=w_sb["wvt"],
                   r=xsp[:, r0:r0 + NR, c0:c0 + NC_]:
                nc.tensor.matmul(o, l, r, start=True, stop=True),
            "sp")

    def emit_vepi(b0):
        # pair epilogue: blocks b0, b0+1 land on adjacent banks by
        # construction of the claim order
        eng = next_epi_eng()
        bank = vbank[b0]
        assert vbank[b0 + 1] == bank + 1
        wait(eng, BALL)
        wait(eng, mark[("vmm", b0 + 1)])
        m = emit_epi(eng, vn16[:, b0:b0 + 2, 0:NN],
                     ps[:, bank:bank + 2, 0:NN], b_sb["bv"])
        mark[("vepi", b0)] = m
        mark[("vepi", b0 + 1)] = m
        bankA[bank] = m
        bankA[bank + 1] = m

    def emit_vt(g):
        wait("sync", MEMSETS)
        for b in range(4 * g, 4 * g + 4):
            wait("sync", mark[("vepi", b)])
        mark[("vt", g)] = op(
            "sync",
            lambda o=vt16[:, 4 * g:4 * g + 4], i_=vn16[:, 4 * g:4 * g + 4, :]:
                nc.sync.dma_start(out=o, in_=i_, transpose=True),
            "sdvt", 16)

    # ---- conv chunks ----
    def chunk_cols(cname, j):
        tot = HALO * W if cname in ("k1", "k2") else NPIX
        return min(CH, tot - j * CH)

    CONV_W = {"k1": "wk1t", "q1": "wq1t", "k2": "wk2t", "q2": "wq2t"}
    CONV_B = {"k1": "bk1", "q1": "bq1", "k2": "bk2", "q2": "bq2"}
    cbank = {}

    def emit_conv(cname, j):
        n = chunk_cols(cname, j)
        nrows = n // W
        bank = claim("pe", True)
        cbank[(cname, j)] = bank
        if cname == "k1":
            if j == 0:
                wait("pe", WALLA)
            xdeps("pe", 4 * j, 4 * j + nrows)
            rhs = xsp[:, 4 * j:4 * j + nrows, PAD:PAD + W]
        elif cname == "q1":
            xdeps("pe", PAD + 4 * j, PAD + 4 * j + nrows)
            rhs = xsp[:, PAD + 4 * j:PAD + 4 * j + nrows, PAD:PAD + W]
        elif cname == "k2":
            if j == 0:
                wait("pe", WALLB)
            wait("pe", mark[("cepi", ("k1", j))])
            rhs = k1[:, bass.ds(j * CH, n)]
        else:
            wait("pe", mark[("cepi", ("q1", j))])
            rhs = q1[:, bass.ds(j * CH, n)]
        mark[("cmm", (cname, j))] = op(
            "pe",
            lambda o=ps[:, bank, :n], l=w_sb[CONV_W[cname]], r=rhs:
                nc.tensor.matmul(o, l, r, start=True, stop=True),
            "sp")

    def emit_cepi(cname, j):
        eng = next_epi_eng()
        bank = cbank[(cname, j)]
        n = chunk_cols(cname, j)
        nrows = n // W
        wait(eng, BALL)
        wait(eng, mark[("cmm", (cname, j))])
        src = ps[:, bank, :n]
        if cname == "k1":
            dst = k1[:, bass.ds(j * CH, n)]
        elif cname == "q1":
            dst = q1[:, bass.ds(j * CH, n)]
        elif cname == "k2":
            dst = kpad[:, 4 * j:4 * j + nrows, PAD:PAD + W]
            src = src.rearrange("p (r w) -> p r w", w=W)
        else:
            br = j // 2
            rlo = (j % 2) * 4
            dst = q[:, br * 8:br * 8 + 8,
                    rlo * BC:(rlo + 4) * BC].rearrange(
                        "p b (r w) -> p b r w", w=BC)
            src = src.rearrange("p (r b w) -> p b r w", r=4, w=BC)
        m = emit_epi(eng, dst, src, b_sb[CONV_B[cname]])
        mark[("cepi", (cname, j))] = m
        bankA[bank] = m
        bankB[bank] = m

    def emit_cepi_pair(cname, j0):
        # chunks j0, j0+1 on adjacent banks (claim-order invariant)
        eng = next_epi_eng()
        bank = cbank[(cname, j0)]
        assert cbank[(cname, j0 + 1)] == bank + 1
        assert chunk_cols(cname, j0) == CH and chunk_cols(cname, j0 + 1) == CH
        wait(eng, BALL)
        wait(eng, mark[("cmm", (cname, j0 + 1))])
        src = ps[:, bank:bank + 2, :].rearrange("p a b -> p (a b)")
        if cname == "k1":
            dst = k1[:, bass.ds(j0 * CH, 2 * CH)]
        elif cname == "q1":
            dst = q1[:, bass.ds(j0 * CH, 2 * CH)]
        elif cname == "k2":
            dst = kpad[:, 4 * j0:4 * j0 + 8, PAD:PAD + W]
            src = src.rearrange("p (r w) -> p r w", w=W)
        else:   # q2 pair covers all 8 rows of one block-row
            br = j0 // 2
            dst = q[:, br * 8:br * 8 + 8, :].rearrange(
                "p b (r w) -> p r b w", w=BC)
            src = src.rearrange("p (r b w) -> p r b w", r=8, w=BC)
        m = emit_epi(eng, dst, src, b_sb[CONV_B[cname]])
        for j in (j0, j0 + 1):
            mark[("cepi", (cname, j))] = m
        bankA[bank] = m
        bankA[bank + 1] = m
        bankB[bank] = m
        bankB[bank + 1] = m

    # ---- attention ----
    sbank = {}

    def emit_S(b):
        r0, c0 = blk_geom(b)
        bank = claim("pe", False)
        sbank[b] = bank
        for j in range(r0 // 4, (r0 + NR + 3) // 4):
            if j < 6:
                wait("pe", mark[("cepi", ("k2", j))])
        for j in ((0, 1) if b < 8 else (2, 3)):
            wait("pe", mark[("cepi", ("q2", j))])
        if b == 0:
            wait("pe", SMASK)
        mark[("smm", b)] = op(
            "pe",
            lambda o=ps[:, bank, 0:NN], l=q[:, b, :],
                   r=kpad[:, r0:r0 + NR, c0:c0 + NC_]:
                nc.tensor.matmul(o, l, r, start=True, stop=False),
            "sp")
        mark[("mmm", b)] = op(
            "pe",
            lambda o=ps[:, bank, 0:NN], l=w_sb["id"], r=maskM[:, _vmap(b), :]:
                nc.tensor.matmul(o, l, r, start=False, stop=True),
            "sp")

    def emit_exp(b):
        s = b % 8
        bank = sbank[b]
        if b == 0:
            wait("act", ESHIFT)
        wait("act", mark[("mmm", b)])
        if b >= 8:
            wait("act", mark[("norm", b - 8)])
        m = op("act",
               lambda o=e8[:, s, :], i_=ps[:, bank, 0:NN], sh=eshift,
                      acc=z16[:, b:b + 1]:
                   nc.scalar.activation(o, i_, EXP, bias=sh, accum_out=acc),
               "sa")
        mark[("exp", b)] = m
        bankA[bank] = m

    def emit_zchain(b0):   # oobc add + recip, pair, DVE (z via exp accum)
        wait("dve", mark[("exp", b0)])
        wait("dve", mark[("exp", b0 + 1)])
        if b0 == 0:
            wait("dve", OOBC)
        zm = op("dve",
                lambda o=z16[:, b0:b0 + 2], i_=z16[:, b0:b0 + 2],
                       i1=oobc16[:, b0:b0 + 2]:
                    nc.vector.tensor_add(o, i_, i1),
                "sv")
        wait("dve", zm)
        rm = op("dve",
                lambda o=rz16[:, b0:b0 + 2], i_=z16[:, b0:b0 + 2]:
                    nc.vector.reciprocal(o, i_),
                "sv")
        mark[("rz", b0)] = rm

    def emit_norm(b):
        s = b % 8
        g = b // 4
        eng = NORM_ENG[b]
        wait(eng, mark[("rz", b - b % 2)])
        if g >= 2:
            wait(eng, mark[("at", g - 2)])
        if eng == "gp":
            wait(eng, MEMSETS)   # keep gp stream ordered anyway
            mark[("norm", b)] = op(
                "gp",
                lambda o=am8[:, s, 0:NN], i_=e8[:, s, :], sc=rz16[:, b:b + 1]:
                    nc.gpsimd.tensor_scalar_mul(o, i_, sc),
                "sg")
        else:
            mark[("norm", b)] = op(
                "dve",
                lambda o=am8[:, s, 0:NN], i_=e8[:, s, :], sc=rz16[:, b:b + 1]:
                    nc.vector.tensor_scalar_mul(o, i_, sc),
                "sv")

    def emit_at(g):
        s0 = (4 * g) % 8
        for b in range(4 * g, 4 * g + 4):
            wait("sync", mark[("norm", b)])
        if g == 0:
            wait("sync", MEMSETS)
        if g >= 2:
            wait("sync", mark[("avmm", 4 * (g - 2) + 3)])
        mark[("at", g)] = op(
            "sync",
            lambda o=at8[:, s0:s0 + 4], i_=am8[:, s0:s0 + 4, :]:
                nc.sync.dma_start(out=o, in_=i_, transpose=True),
            "sdat", 16)

    def emit_av(b):
        s = b % 8
        g = b // 4
        bank = sbank[b]
        wait("pe", bankB[bank])
        wait("pe", mark[("vt", g)])
        wait("pe", mark[("at", g)])
        for ch in range(3):
            mark[("avmm", b)] = op(
                "pe",
                lambda o=ps[:, bank, NN2:CH], l=vt16[:, b, ch, :],
                       r=at8[:, s, ch, :], st=(ch == 0), sp_=(ch == 2):
                    nc.tensor.matmul(o, l, r, start=st, stop=sp_),
                "sp")

    def emit_acopy(b):   # pair: blocks b, b+1 on adjacent banks
        r0, c0 = blk_geom(b)
        bank = sbank[b]
        assert sbank[b + 1] == bank + 1
        eng = next_epi_eng()
        wait(eng, mark[("avmm", b + 1)])
        dst = attn[:, r0:r0 + BR, c0:c0 + 2 * BC].rearrange(
            "p r (a w) -> p a r w", w=BC)
        src = ps[:, bank:bank + 2, NN2:CH].rearrange(
            "p a (r w) -> p a r w", w=BC)
        if eng == "act":
            m = op("act", lambda o=dst, i_=src: nc.scalar.copy(o, i_), "sa")
        else:
            m = op("dve",
                   lambda o=dst, i_=src: nc.vector.tensor_scalar(
                       o, i_, 0.0, None, ADD),
                   "sv")
        mark[("acopy", b)] = m
        bankB[bank] = m
        bankB[bank + 1] = m

    def emit_avpair(b0):
        emit_av(b0)
        emit_av(b0 + 1)
        emit_acopy(b0)

    # ---- output conv: column chunks, rolling banks ----
    def emit_o(i):
        bank = claim("pe", True)
        wait("pe", mark[("acopy", 2 * i)])
        wait("pe", mark[("acopy", 8 + 2 * i)])
        op("pe",
           lambda o=ps[:, bank, :], l=w_sb["wat"],
                  r=attn[:, :, 32 * i:32 * i + 32]:
               nc.tensor.matmul(o, l, r, start=True, stop=False),
           "sp")
        om = op("pe",
                lambda o=ps[:, bank, :], l=w_sb["wxt"],
                       r=xsp[:, PAD:PAD + RPC, PAD + 32 * i:PAD + 32 * i + 32]:
                    nc.tensor.matmul(o, l, r, start=False, stop=True),
                "sp")
        eng = next_epi_eng()
        wait(eng, om)
        if i >= 2:
            wait(eng, mark[("odma", i - 2)])
        m = emit_epi(eng, yt[:, i % 2, :], ps[:, bank, :], b_sb["bo"],
                     relu=False)
        mark[("oepi", i)] = m
        bankA[bank] = m
        bankB[bank] = m
        wait("sync", m)
        mark[("odma", i)] = op(
            "sync",
            lambda o=y_d[:, bass.ts(i, CH)], i_=yt[:, i % 2, :]:
                nc.sync.dma_start(out=o, in_=i_),
            "sdout", 16)

    # =====================================================================
    # Emission schedule
    def emit_S_quad(g):
        for b in range(4 * g, 4 * g + 4):
            emit_S(b)
            emit_exp(b)
        for p in (0, 2):
            b0 = 4 * g + p
            emit_zchain(b0)
            emit_norm(b0)
            emit_norm(b0 + 1)

    # v4.0-style skeleton: dense conv+v phase, then S quads, then AV pairs.
    emit_conv("k1", 0)
    for b in range(4):
        emit_v(b)
        if b % 2:
            emit_vepi(b - 1)
    emit_conv("k1", 1)
    emit_cepi("k1", 0)
    for b in range(4, 8):
        emit_v(b)
        if b % 2:
            emit_vepi(b - 1)
    emit_vt(0)
    emit_conv("k1", 2)
    emit_cepi("k1", 1)
    emit_conv("k1", 3)
    emit_cepi("k1", 2)
    for b in range(8, 12):
        emit_v(b)
        if b % 2:
            emit_vepi(b - 1)
    emit_vt(1)
    emit_conv("k1", 4)
    emit_cepi("k1", 3)
    emit_conv("k1", 5)
    emit_cepi("k1", 4)
    for b in range(12, 16):
        emit_v(b)
        if b % 2:
            emit_vepi(b - 1)
    emit_vt(2)
    emit_conv("q1", 0)
    emit_cepi("k1", 5)
    emit_cepi("q1", 0)
    for j in range(1, 4):
        emit_conv("q1", j)
        emit_cepi("q1", j)
    emit_vt(3)
    emit_conv("k2", 4)
    emit_cepi("k2", 4)
    emit_conv("k2", 5)
    emit_cepi("k2", 5)
    emit_conv("k2", 0)
    emit_cepi("k2", 0)
    emit_conv("k2", 1)
    emit_cepi("k2", 1)
    emit_conv("k2", 2)
    emit_cepi("k2", 2)
    emit_conv("k2", 3)
    emit_cepi("k2", 3)
    emit_conv("q2", 0)
    emit_cepi("q2", 0)
    emit_conv("q2", 1)
    emit_cepi("q2", 1)
    emit_S_quad(0)
    emit_at(0)
    emit_conv("q2", 2)
    emit_cepi("q2", 2)
    emit_conv("q2", 3)
    emit_cepi("q2", 3)
    emit_S_quad(1)
    emit_at(1)
    emit_S_quad(2)
    emit_S_quad(3)
    emit_avpair(0)
    emit_avpair(2)
    emit_at(2)
    emit_avpair(4)
    emit_avpair(6)
    emit_at(3)
    emit_avpair(8)
    emit_avpair(10)
    emit_o(0)
    emit_o(1)
    emit_avpair(12)
    emit_avpair(14)
    emit_o(2)
    emit_o(3)

    if DEBUG_OUTS:
        dbg = {
            "d_q": q.rearrange("p a b -> p (a b)"),
            "d_kpad": kpad.rearrange("p r w -> p (r w)"),
            "d_vn": vn16.rearrange("p a b -> p (a b)"),
            "d_vt": vt16.rearrange("p a b c -> p (a b c)"),
            "d_attn": attn.rearrange("p r w -> p (r w)"),
            "d_z": z16,
            "d_rz": rz16,
            "d_am": am8.rearrange("p a b -> p (a b)"),
            "d_at": at8.rearrange("p a b c -> p (a b c)"),
        }
        for nm, src in dbg.items():
            dd = nc.dram_tensor(nm, list(src.shape),
                                src.dtype, kind="ExternalOutput").ap()
            for s_ in ("sp", "sa", "sv", "sg"):
                wait("sync", (s_, cnt[s_]))
            op("sync", lambda o=dd, i_=src: nc.sync.dma_start(out=o, in_=i_),
               "sdout", 16)

    # ---- tail barrier ----
    for s_ in ("sp", "sa", "sv", "sg", "sdvt", "sdat", "sdout",
               "sdwa", "sdwb", "sdball", "sdo", "sdm"):
        wait("sync", (s_, cnt[s_]))
    for j in range(XCH):
        wait("sync", (f"sdx{j}", cnt[f"sdx{j}"]))

    # ---- emit ----
    def run(eng_name, eng_obj):
        hwm = {}
        for item in plan[eng_name]:
            if item[0] == "w":
                _, s_, v = item
                if hwm.get(s_, 0) >= v:
                    continue
                hwm[s_] = v
                eng_obj.wait_ge(sems[s_], v)
            else:
                _, fn, s_, inc = item
                inst = fn()
                if s_:
                    inst.then_inc(sems[s_], inc)

    with nc.Block() as block:
        @block.sync
        def _(e):
            run("sync", e)

        @block.tensor
        def _(e):
            run("pe", e)

        @block.scalar
        def _(e):
            run("act", e)

        @block.vector
        def _(e):
            run("dve", e)

        @block.gpsimd
        def _(e):
            run("gp", e)

    return nc


_PROGRAM = None


def _host_inputs(x, w_q1, s_q1, b_q1, w_q2, s_q2, b_q2,
                 w_k1, s_k1, b_k1, w_k2, s_k2, b_k2,
                 w_v, s_v, b_v, w_o, s_o, b_o):
    def foldT(w, s):
        return np.ascontiguousarray((s[:, None] * w).T.astype(ml_dtypes.bfloat16))

    wq1t, wq2t = foldT(w_q1, s_q1), foldT(w_q2, s_q2)
    wk1t, wk2t = foldT(w_k1, s_k1), foldT(w_k2, s_k2)
    wvt = foldT(w_v, s_v)
    wo = s_o[:, None] * w_o
    wat = np.ascontiguousarray(wo[:, :C].T.astype(ml_dtypes.bfloat16))
    wxt = np.ascontiguousarray(wo[:, C:].T.astype(ml_dtypes.bfloat16))

    col = lambda b: np.ascontiguousarray(b.astype(np.float32)[:, None])

    valid = np.zeros((BR * BC, NR, NC_), bool)
    for r in range(BR):
        for c in range(BC):
            p = r * BC + c
            valid[p, r:r + 7, c:c + 7] = True

    X = np.asarray(x, np.float32).reshape(C, H, W)
    wallA = np.concatenate([wk1t, wvt, wq1t], axis=1)
    wallB = np.concatenate(
        [wk2t, wq2t, wat, wxt, np.eye(C, dtype=ml_dtypes.bfloat16)], axis=1)
    shared = dict(walla=np.ascontiguousarray(wallA),
                  wallb=np.ascontiguousarray(wallB))

    e16v = np.float32(np.exp(EXP_SHIFT))
    var_rep = {rc * 3 + cc: (rc, {0: 0, 1: 3, 2: 7}[cc])
               for rc in range(2) for cc in range(3)}

    in_maps = []
    for core in range(NCORES):
        h0 = core * RPC
        xsb = np.zeros((C, HALO, WP), np.float32)
        lo, hi = h0 - PAD, h0 + RPC + PAD
        slo, shi = max(lo, 0), min(hi, H)
        xsb[:, slo - lo:shi - lo, PAD:PAD + W] = X[:, slo:shi]

        maskm = np.empty((NVAR, BR * BC, NN), np.float32)
        for v, (brr, cb) in var_rep.items():
            rowok = np.array([0 <= h0 + brr * BR + ri - PAD < H
                              for ri in range(NR)])
            colok = np.array([0 <= cb * BC + ci - PAD < W
                              for ci in range(NC_)])
            inimg = rowok[:, None] & colok[None, :]
            mb = np.where(valid & inimg[None, :, :], 0.0, MASKV)
            maskm[v] = mb.reshape(BR * BC, NN)
        oobc = np.empty((16, BR * BC), np.float32)
        for b in range(16):
            brr, cb = b // 8, b % 8
            rowok = np.array([0 <= h0 + brr * BR + ri - PAD < H
                              for ri in range(NR)])
            colok = np.array([0 <= cb * BC + ci - PAD < W
                              for ci in range(NC_)])
            inimg = rowok[:, None] & colok[None, :]
            n_oob = (valid & ~inimg[None, :, :]).sum(axis=(1, 2))
            oobc[b] = n_oob * e16v
        m = dict(shared)
        m["xs"] = np.ascontiguousarray(
            xsb.reshape(C, NXP).astype(ml_dtypes.bfloat16))
        m["smask"] = np.ascontiguousarray(
            maskm.transpose(1, 0, 2).reshape(BR * BC, NVAR * NN)
            .astype(ml_dtypes.bfloat16))
        m["oobc"] = np.ascontiguousarray(oobc.T.astype(np.float32))
        m["ball"] = np.ascontiguousarray(np.concatenate(
            [col(b_q1), col(b_q2), col(b_k1), col(b_k2), col(b_v),
             col(b_o), np.zeros((C, 2), np.float32)], axis=1))
        in_maps.append(m)
    return in_maps


def kernel(**inputs):
    global _PROGRAM
    if _PROGRAM is None:
        _PROGRAM = _build_program()
    in_maps = _host_inputs(**{k: np.asarray(v) for k, v in inputs.items()})
    res = run_bass_kernel_spmd(_PROGRAM, in_maps, core_ids=list(range(NCORES)))
    stripes = [np.asarray(r["y"]).astype(np.float32)
               .reshape(C, 4, RPC, 32).transpose(0, 2, 1, 3).reshape(C, RPC, W)
               for r in res.results]
    return np.concatenate(stripes, axis=1).reshape(1, C, H, W)


if __name__ == "__main__":
    rng = np.random.default_rng(0)
    fake = {"x": rng.standard_normal((1, C, H, W)).astype(np.float32)}
    for n in ("q1", "q2", "k1", "k2", "v", "o"):
        cin = 2 * C if n == "o" else C
        fake["w_" + n] = (rng.standard_normal((C, cin)) / np.sqrt(cin)).astype(np.float32)
        fake["s_" + n] = rng.uniform(0.5, 1.5, C).astype(np.float32)
        fake["b_" + n] = (rng.standard_normal(C) * 0.1).astype(np.float32)
    out = kernel(**fake)
    print("kernel output", out.shape, out.dtype)


# revision 33
# speedup vs baseline: 1.2793x; 1.0184x over previous
"""Trainium2 Bass kernel for ModalitySpecificLocalSelfAttention (7x7 window).

Spatial-parallel over H across 8 cores (16-row stripe + 3-row halo each).
v4.1 design (49.9us; v4 52.9us; v3 61.5us). Pair-granular v epilogues
(adjacent PSUM banks, one op per 2 blocks); conv epilogues measured
cheaper as singles (DVE 2x mode) so they stay per-chunk.
  - Rolling PSUM bank tracker (regions [0:308] for v/S/conv, [384:512]
    for AV) replaces v3's hand-woven tenancy waits; emission order makes
    every exp/acopy pair land on adjacent banks.
  - Dense conv+v phase (v blocks interleaved with k1 chunks as xs DMA
    chunks land), then S quads with per-block exp + accum_out (z produced
    by the ACT pass; kills v3's 6.3us of DVE reduces), then AV pairs.
  - smask deduped to 6 variants (row-class x col-class) instead of 16
    block masks: input drops 790KB of DMA.
  - Weight DMA split: wallA (wk1t/wvt/wq1t) first on the sync ring so the
    first matmul starts ~0.7us earlier; wallB rides the scalar ring.
  - All transposes quad-batched on the sync ring; act-table load warmed
    by a dummy op at t=0 so its 1.3us hides under the input DMAs.
  CAUTION: removing the exp->norm slot-pacing waits or widening the
  softmax buffers to 16 slots hard-crashes the device (deterministic
  INTERNAL error; suspected cayman event-accel deadlock from unspaced
  multi-engine then_inc streams). Keep the 8-slot structure.
"""

import sys

for _p in ("/opt/trn_rl_repo", "/root/.axon_site/_ro/trn_rl_repo"):
    if _p not in sys.path:
        sys.path.append(_p)

import ml_dtypes
import numpy as np

import concourse.bass as bass
from concourse import mybir
from concourse.bass_utils import run_bass_kernel_spmd

F32 = mybir.dt.float32
BF16 = mybir.dt.bfloat16

C = 128
H = 128
W = 128
NCORES = 8
RPC = H // NCORES          # 16 rows per core
PAD = 3
HALO = RPC + 2 * PAD       # 22 rows incl halo
WP = W + 2 * PAD           # 134 padded width
BR, BC = 8, 16             # pixel block 8 rows x 16 cols
NR, NC_ = BR + 2 * PAD, BC + 2 * PAD   # 14 x 22 neighborhood
NN = NR * NC_              # 308
NN2 = 384                  # padded to 3x128 for the xbar transpose
NPIX = RPC * W             # 2048
NXP = HALO * WP            # 2948 padded stripe pixels
CH = 512
NVAR = 6                   # mask variants: rowclass(2) x colclass(3)
EXP_SHIFT = -16.0
MASKV = -40.0
DEBUG_OUTS = False

# xs input chunking (4 chunks over the flattened [C, NXP])
XCH = 4
XSPL = [0, 737, 1474, 2211, NXP]

# engine per softmax-normalize block (gp = GPSIMD is ~15ns/elem — useless;
# measured 4.6us per 308-elem norm)
NORM_ENG = ["dve"] * 16

RELU = mybir.ActivationFunctionType.Relu
IDENT = mybir.ActivationFunctionType.Identity
EXP = mybir.ActivationFunctionType.Exp
COPY = mybir.ActivationFunctionType.Copy
ADD = mybir.AluOpType.add
MULT = mybir.AluOpType.mult
MAXOP = mybir.AluOpType.max


def _vmap(b):
    """block index -> mask variant slot"""
    rc = b // 8
    cb = b % 8
    cc = 0 if cb == 0 else (2 if cb == 7 else 1)
    return rc * 3 + cc


def _build_program():
    nc = bass.Bass("TRN2", target_bir_lowering=False, debug=False)

    # ---- DRAM I/O ----
    xs_d = nc.dram_tensor("xs", [C, NXP], BF16, kind="ExternalInput").ap()
    wa_d = nc.dram_tensor("walla", [C, 3 * C], BF16, kind="ExternalInput").ap()
    wb_d = nc.dram_tensor("wallb", [C, 5 * C], BF16, kind="ExternalInput").ap()
    ball_d = nc.dram_tensor("ball", [C, 8], F32, kind="ExternalInput").ap()
    smask_d = nc.dram_tensor("smask", [C, NVAR * NN], BF16,
                             kind="ExternalInput").ap()
    oobc_d = nc.dram_tensor("oobc", [C, 16], F32, kind="ExternalInput").ap()
    y_d = nc.dram_tensor("y", [C, NPIX], BF16, kind="ExternalOutput").ap()

    # ---- SBUF ----
    sb = lambda name, shape, dt: nc.alloc_sbuf_tensor(name, list(shape), dt).ap()
    xsp = sb("xsp_sb", [C, HALO, WP], BF16)
    k1 = sb("k1_sb", [C, HALO * W], BF16)
    q1 = sb("q1_sb", [C, NPIX], BF16)
    q = sb("q_sb", [C, 16, C], BF16)       # block-major
    kpad = sb("kpad_sb", [C, HALO, WP], BF16)
    vn16 = sb("vn16_sb", [C, 16, NN2], BF16)
    vt16 = sb("vt16_sb", [C, 16, 3, C], BF16)
    e8 = sb("e8_sb", [C, 8, NN], BF16)
    am8 = sb("am8_sb", [C, 8, NN2], BF16)
    at8 = sb("at8_sb", [C, 8, 3, C], BF16)
    z16 = sb("z16_sb", [C, 16], F32)
    rz16 = sb("rz16_sb", [C, 16], F32)
    attn = sb("attn_sb", [C, RPC, W], BF16)
    walla = sb("walla_sb", [C, 3, C], BF16)
    wallb = sb("wallb_sb", [C, 5, C], BF16)
    ball = sb("ball_sb", [C, 8], F32)
    maskM = sb("maskM_sb", [C, NVAR, NN], BF16)
    oobc16 = sb("oobc16_sb", [C, 16], F32)
    eshift = sb("eshift_sb", [C, 1], F32)
    warm = sb("warm_sb", [C, 1], F32)
    yt = sb("yt_sb", [C, 2, CH], BF16)

    WA_IDX = {"wk1t": 0, "wvt": 1, "wq1t": 2}
    WB_IDX = {"wk2t": 0, "wq2t": 1, "wat": 2, "wxt": 3, "id": 4}
    w_sb = {n: walla[:, k, :] for n, k in WA_IDX.items()}
    w_sb.update({n: wallb[:, k, :] for n, k in WB_IDX.items()})
    B_IDX = {n: k for k, n in enumerate(
        ("bq1", "bq2", "bk1", "bk2", "bv", "bo"))}
    b_sb = {n: ball[:, k:k + 1] for n, k in B_IDX.items()}

    ps = nc.alloc_psum_tensor("ps", [C, 8, CH], F32).ap()

    # ---- semaphores / plan ----
    sem_names = ("sdwa", "sdwb", "sdball", "sdo", "sdm", "sp", "sa", "sv",
                 "sg", "sdvt", "sdat", "sdout") + tuple(
                     f"sdx{j}" for j in range(XCH))
    sems = {n: nc.alloc_semaphore(n) for n in sem_names}
    ENGS = ("sync", "pe", "act", "dve", "gp")
    plan = {e: [] for e in ENGS}
    cnt = {n: 0 for n in sem_names}

    def op(eng, fn, sem, inc=1):
        plan[eng].append(("op", fn, sem, inc))
        if sem:
            cnt[sem] += inc
            return (sem, cnt[sem])
        return None

    def wait(eng, mark):
        if mark is not None:
            sem, val = mark
            if val and val > 0:
                plan[eng].append(("w", sem, val))

    # ---- init: GP memsets, DVE eshift, ACT table warmup ----
    for c0 in (0, WP - PAD):
        op("gp", lambda tf=kpad[:, :, c0:c0 + PAD]: nc.gpsimd.memset(tf, 0.0),
           "sg")
    op("gp", lambda: nc.gpsimd.memset(vn16[:, :, NN:NN2], 0.0), "sg")
    op("gp", lambda: nc.gpsimd.memset(am8[:, :, NN:NN2], 0.0), "sg")
    MEMSETS = ("sg", cnt["sg"])
    ESHIFT = op("dve", lambda: nc.vector.memset(eshift, EXP_SHIFT), "sv")
    wait("act", ESHIFT)
    op("act", lambda: nc.scalar.activation(warm, eshift, RELU), "sa")

    # ---- input DMAs ----
    def dma(eng, sem, dst, srcd):
        fn = nc.sync.dma_start if eng == "sync" else nc.scalar.dma_start
        return op(eng, lambda d=dst, s=srcd, f=fn: f(out=d, in_=s), sem, 16)

    xsp_f = xsp.rearrange("p r w -> p (r w)")
    XS = [None] * XCH

    def xs_dma(eng, i):
        lo, hi = XSPL[i], XSPL[i + 1]
        XS[i] = dma(eng, f"sdx{i}", xsp_f[:, lo:hi], xs_d[:, lo:hi])

    WALLA = dma("sync", "sdwa", walla.rearrange("p a b -> p (a b)"), wa_d)
    xs_dma("sync", 0)
    xs_dma("act", 1)
    WALLB = dma("act", "sdwb", wallb.rearrange("p a b -> p (a b)"), wb_d)
    xs_dma("sync", 2)
    xs_dma("sync", 3)
    BALL = dma("act", "sdball", ball, ball_d)
    OOBC = dma("act", "sdo", oobc16, oobc_d)
    SMASK = dma("sync", "sdm", maskM.rearrange("p a b -> p (a b)"), smask_d)

    def xdeps(eng, row0, row1):
        c0 = 0
        c1 = XCH - 1
        for c in range(XCH):
            if XSPL[c + 1] > row0 * WP:
                c0 = c
                break
        for c in range(XCH):
            if XSPL[c + 1] >= row1 * WP:
                c1 = c
                break
        for c in range(c0, c1 + 1):
            wait(eng, XS[c])

    mark = {}

    # ---- PSUM tenancy tracker ----
    bankA = [None] * 8       # region [0:308] freeing mark
    bankB = [None] * 8       # region [384:512] freeing mark
    seqA = [0]

    def claim(eng, full, bank=None):
        if bank is None:
            bank = seqA[0] % 8
            seqA[0] += 1
        wait(eng, bankA[bank])
        if full:
            wait(eng, bankB[bank])
        return bank

    epi_rr = [0]

    def next_epi_eng():
        epi_rr[0] ^= 1
        return "dve" if epi_rr[0] else "act"

    def emit_epi(eng, dst, src, bias, relu=True):
        if eng == "act":
            return op("act",
                      lambda o=dst, i_=src, b_=bias, f=(RELU if relu else IDENT):
                          nc.scalar.activation(o, i_, f, bias=b_),
                      "sa")
        return op("dve",
                  lambda o=dst, i_=src, b_=bias,
                         op1=(MAXOP if relu else mybir.AluOpType.bypass):
                      nc.vector.tensor_scalar(o, i_, b_, 0.0, ADD, op1),
                  "sv")

    # ---- v blocks ----
    def blk_geom(b):
        return 8 * (b // 8), (b % 8) * 16

    vbank = {}

    def emit_v(b):
        r0, c0 = blk_geom(b)
        bank = claim("pe", False)
        vbank[b] = bank
        xdeps("pe", r0, r0 + NR)
        mark[("vmm", b)] = op(
            "pe",
            lambda o=ps[:, bank, 0:NN], l=w_sb["wvt"],
                   r=xsp[:, r0:r0 + NR, c0:c0 + NC_]:
                nc.tensor.matmul(o, l, r, start=True, stop=True),
            "sp")

    def emit_vepi(b0):
        # pair epilogue: blocks b0, b0+1 land on adjacent banks by
        # construction of the claim order
        eng = next_epi_eng()
        bank = vbank[b0]
        assert vbank[b0 + 1] == bank + 1
        wait(eng, BALL)
        wait(eng, mark[("vmm", b0 + 1)])
        m = emit_epi(eng, vn16[:, b0:b0 + 2, 0:NN],
                     ps[:, bank:bank + 2, 0:NN], b_sb["bv"])
        mark[("vepi", b0)] = m
        mark[("vepi", b0 + 1)] = m
        bankA[bank] = m
        bankA[bank + 1] = m

    def emit_vt(g):
        wait("sync", MEMSETS)
        for b in range(4 * g, 4 * g + 4):
            wait("sync", mark[("vepi", b)])
        mark[("vt", g)] = op(
            "sync",
            lambda o=vt16[:, 4 * g:4 * g + 4], i_=vn16[:, 4 * g:4 * g + 4, :]:
                nc.sync.dma_start(out=o, in_=i_, transpose=True),
            "sdvt", 16)

    # ---- conv chunks ----
    def chunk_cols(cname, j):
        tot = HALO * W if cname in ("k1", "k2") else NPIX
        return min(CH, tot - j * CH)

    CONV_W = {"k1": "wk1t", "q1": "wq1t", "k2": "wk2t", "q2": "wq2t"}
    CONV_B = {"k1": "bk1", "q1": "bq1", "k2": "bk2", "q2": "bq2"}
    cbank = {}

    def emit_conv(cname, j):
        n = chunk_cols(cname, j)
        nrows = n // W
        bank = claim("pe", True)
        cbank[(cname, j)] = bank
        if cname == "k1":
            if j == 0:
                wait("pe", WALLA)
            xdeps("pe", 4 * j, 4 * j + nrows)
            rhs = xsp[:, 4 * j:4 * j + nrows, PAD:PAD + W]
        elif cname == "q1":
            xdeps("pe", PAD + 4 * j, PAD + 4 * j + nrows)
            rhs = xsp[:, PAD + 4 * j:PAD + 4 * j + nrows, PAD:PAD + W]
        elif cname == "k2":
            if j == 0:
                wait("pe", WALLB)
            wait("pe", mark[("cepi", ("k1", j))])
            rhs = k1[:, bass.ds(j * CH, n)]
        else:
            wait("pe", mark[("cepi", ("q1", j))])
            rhs = q1[:, bass.ds(j * CH, n)]
        mark[("cmm", (cname, j))] = op(
            "pe",
            lambda o=ps[:, bank, :n], l=w_sb[CONV_W[cname]], r=rhs:
                nc.tensor.matmul(o, l, r, start=True, stop=True),
            "sp")

    def emit_cepi(cname, j):
        eng = next_epi_eng()
        bank = cbank[(cname, j)]
        n = chunk_cols(cname, j)
        nrows = n // W
        wait(eng, BALL)
        wait(eng, mark[("cmm", (cname, j))])
        src = ps[:, bank, :n]
        if cname == "k1":
            dst = k1[:, bass.ds(j * CH, n)]
        elif cname == "q1":
            dst = q1[:, bass.ds(j * CH, n)]
        elif cname == "k2":
            dst = kpad[:, 4 * j:4 * j + nrows, PAD:PAD + W]
            src = src.rearrange("p (r w) -> p r w", w=W)
        else:
            br = j // 2
            rlo = (j % 2) * 4
            dst = q[:, br * 8:br * 8 + 8,
                    rlo * BC:(rlo + 4) * BC].rearrange(
                        "p b (r w) -> p b r w", w=BC)
            src = src.rearrange("p (r b w) -> p b r w", r=4, w=BC)
        m = emit_epi(eng, dst, src, b_sb[CONV_B[cname]])
        mark[("cepi", (cname, j))] = m
        bankA[bank] = m
        bankB[bank] = m

    def emit_cepi_pair(cname, j0):
        # chunks j0, j0+1 on adjacent banks (claim-order invariant)
        eng = next_epi_eng()
        bank = cbank[(cname, j0)]
        assert cbank[(cname, j0 + 1)] == bank + 1
        assert chunk_cols(cname, j0) == CH and chunk_cols(cname, j0 + 1) == CH
        wait(eng, BALL)
        wait(eng, mark[("cmm", (cname, j0 + 1))])
        src = ps[:, bank:bank + 2, :].rearrange("p a b -> p (a b)")
        if cname == "k1":
            dst = k1[:, bass.ds(j0 * CH, 2 * CH)]
        elif cname == "q1":
            dst = q1[:, bass.ds(j0 * CH, 2 * CH)]
        elif cname == "k2":
            dst = kpad[:, 4 * j0:4 * j0 + 8, PAD:PAD + W]
            src = src.rearrange("p (r w) -> p r w", w=W)
        else:   # q2 pair covers all 8 rows of one block-row
            br = j0 // 2
            dst = q[:, br * 8:br * 8 + 8, :].rearrange(
                "p b (r w) -> p r b w", w=BC)
            src = src.rearrange("p (r b w) -> p r b w", r=8, w=BC)
        m = emit_epi(eng, dst, src, b_sb[CONV_B[cname]])
        for j in (j0, j0 + 1):
            mark[("cepi", (cname, j))] = m
        bankA[bank] = m
        bankA[bank + 1] = m
        bankB[bank] = m
        bankB[bank + 1] = m

    # ---- attention ----
    sbank = {}

    def emit_S(b):
        r0, c0 = blk_geom(b)
        bank = claim("pe", False)
        sbank[b] = bank
        for j in range(r0 // 4, (r0 + NR + 3) // 4):
            if j < 6:
                wait("pe", mark[("cepi", ("k2", j))])
        for j in ((0, 1) if b < 8 else (2, 3)):
            wait("pe", mark[("cepi", ("q2", j))])
        if b == 0:
            wait("pe", SMASK)
        mark[("smm", b)] = op(
            "pe",
            lambda o=ps[:, bank, 0:NN], l=q[:, b, :],
                   r=kpad[:, r0:r0 + NR, c0:c0 + NC_]:
                nc.tensor.matmul(o, l, r, start=True, stop=False),
            "sp")
        mark[("mmm", b)] = op(
            "pe",
            lambda o=ps[:, bank, 0:NN], l=w_sb["id"], r=maskM[:, _vmap(b), :]:
                nc.tensor.matmul(o, l, r, start=False, stop=True),
            "sp")

    def emit_exp(b):
        s = b % 8
        bank = sbank[b]
        if b == 0:
            wait("act", ESHIFT)
        wait("act", mark[("mmm", b)])
        if b >= 8:
            wait("act", mark[("norm", b - 8)])
        m = op("act",
               lambda o=e8[:, s, :], i_=ps[:, bank, 0:NN], sh=eshift,
                      acc=z16[:, b:b + 1]:
                   nc.scalar.activation(o, i_, EXP, bias=sh, accum_out=acc),
               "sa")
        mark[("exp", b)] = m
        bankA[bank] = m

    def emit_zchain(b0):   # oobc add + recip, pair, DVE (z via exp accum)
        wait("dve", mark[("exp", b0)])
        wait("dve", mark[("exp", b0 + 1)])
        if b0 == 0:
            wait("dve", OOBC)
        zm = op("dve",
                lambda o=z16[:, b0:b0 + 2], i_=z16[:, b0:b0 + 2],
                       i1=oobc16[:, b0:b0 + 2]:
                    nc.vector.tensor_add(o, i_, i1),
                "sv")
        wait("dve", zm)
        rm = op("dve",
                lambda o=rz16[:, b0:b0 + 2], i_=z16[:, b0:b0 + 2]:
                    nc.vector.reciprocal(o, i_),
                "sv")
        mark[("rz", b0)] = rm

    def emit_norm(b):
        s = b % 8
        g = b // 4
        eng = NORM_ENG[b]
        wait(eng, mark[("rz", b - b % 2)])
        if g >= 2:
            wait(eng, mark[("at", g - 2)])
        if eng == "gp":
            wait(eng, MEMSETS)   # keep gp stream ordered anyway
            mark[("norm", b)] = op(
                "gp",
                lambda o=am8[:, s, 0:NN], i_=e8[:, s, :], sc=rz16[:, b:b + 1]:
                    nc.gpsimd.tensor_scalar_mul(o, i_, sc),
                "sg")
        else:
            mark[("norm", b)] = op(
                "dve",
                lambda o=am8[:, s, 0:NN], i_=e8[:, s, :], sc=rz16[:, b:b + 1]:
                    nc.vector.tensor_scalar_mul(o, i_, sc),
                "sv")

    def emit_at(g):
        s0 = (4 * g) % 8
        for b in range(4 * g, 4 * g + 4):
            wait("sync", mark[("norm", b)])
        if g == 0:
            wait("sync", MEMSETS)
        if g >= 2:
            wait("sync", mark[("avmm", 4 * (g - 2) + 3)])
        mark[("at", g)] = op(
            "sync",
            lambda o=at8[:, s0:s0 + 4], i_=am8[:, s0:s0 + 4, :]:
                nc.sync.dma_start(out=o, in_=i_, transpose=True),
            "sdat", 16)

    def emit_at_pair(b0):   # blocks b0, b0+1 (quads 2,3): earlier issue
        s0 = b0 % 8
        for b in (b0, b0 + 1):
            wait("sync", mark[("norm", b)])
        wait("sync", mark[("avmm", b0 - 7)])   # at8 slot pair re-read
        mark[("atp", b0)] = op(
            "sync",
            lambda o=at8[:, s0:s0 + 2], i_=am8[:, s0:s0 + 2, :]:
                nc.sync.dma_start(out=o, in_=i_, transpose=True),
            "sdat", 16)

    def emit_av(b):
        s = b % 8
        g = b // 4
        bank = sbank[b]
        wait("pe", bankB[bank])
        wait("pe", mark[("vt", g)])
        if b >= 8:
            wait("pe", mark[("atp", b - b % 2)])
        else:
            wait("pe", mark[("at", g)])
        for ch in range(3):
            mark[("avmm", b)] = op(
                "pe",
                lambda o=ps[:, bank, NN2:CH], l=vt16[:, b, ch, :],
                       r=at8[:, s, ch, :], st=(ch == 0), sp_=(ch == 2):
                    nc.tensor.matmul(o, l, r, start=st, stop=sp_),
                "sp")

    def emit_acopy(b):   # pair: blocks b, b+1 on adjacent banks
        r0, c0 = blk_geom(b)
        bank = sbank[b]
        assert sbank[b + 1] == bank + 1
        eng = next_epi_eng()
        wait(eng, mark[("avmm", b + 1)])
        dst = attn[:, r0:r0 + BR, c0:c0 + 2 * BC].rearrange(
            "p r (a w) -> p a r w", w=BC)
        src = ps[:, bank:bank + 2, NN2:CH].rearrange(
            "p a (r w) -> p a r w", w=BC)
        if eng == "act":
            m = op("act", lambda o=dst, i_=src: nc.scalar.copy(o, i_), "sa")
        else:
            m = op("dve",
                   lambda o=dst, i_=src: nc.vector.tensor_scalar(
                       o, i_, 0.0, None, ADD),
                   "sv")
        mark[("acopy", b)] = m
        bankB[bank] = m
        bankB[bank + 1] = m

    def emit_avpair(b0):
        emit_av(b0)
        emit_av(b0 + 1)
        emit_acopy(b0)

    # ---- output conv: column chunks, rolling banks ----
    def emit_o(i):
        bank = claim("pe", True)
        wait("pe", mark[("acopy", 2 * i)])
        wait("pe", mark[("acopy", 8 + 2 * i)])
        op("pe",
           lambda o=ps[:, bank, :], l=w_sb["wat"],
                  r=attn[:, :, 32 * i:32 * i + 32]:
               nc.tensor.matmul(o, l, r, start=True, stop=False),
           "sp")
        om = op("pe",
                lambda o=ps[:, bank, :], l=w_sb["wxt"],
                       r=xsp[:, PAD:PAD + RPC, PAD + 32 * i:PAD + 32 * i + 32]:
                    nc.tensor.matmul(o, l, r, start=False, stop=True),
                "sp")
        eng = next_epi_eng()
        wait(eng, om)
        if i >= 2:
            wait(eng, mark[("odma", i - 2)])
        m = emit_epi(eng, yt[:, i % 2, :], ps[:, bank, :], b_sb["bo"],
                     relu=False)
        mark[("oepi", i)] = m
        bankA[bank] = m
        bankB[bank] = m
        wait("sync", m)
        mark[("odma", i)] = op(
            "sync",
            lambda o=y_d[:, bass.ts(i, CH)], i_=yt[:, i % 2, :]:
                nc.sync.dma_start(out=o, in_=i_),
            "sdout", 16)

    # =====================================================================
    # Emission schedule
    def emit_S_quad(g):
        for b in range(4 * g, 4 * g + 4):
            emit_S(b)
            emit_exp(b)
        for p in (0, 2):
            b0 = 4 * g + p
            emit_zchain(b0)
            emit_norm(b0)
            emit_norm(b0 + 1)

    # v4.0-style skeleton: dense conv+v phase, then S quads, then AV pairs.
    emit_conv("k1", 0)
    for b in range(4):
        emit_v(b)
        if b % 2:
            emit_vepi(b - 1)
    emit_conv("k1", 1)
    emit_cepi("k1", 0)
    for b in range(4, 8):
        emit_v(b)
        if b % 2:
            emit_vepi(b - 1)
    emit_vt(0)
    emit_conv("k1", 2)
    emit_cepi("k1", 1)
    emit_conv("k1", 3)
    emit_cepi("k1", 2)
    for b in range(8, 12):
        emit_v(b)
        if b % 2:
            emit_vepi(b - 1)
    emit_vt(1)
    emit_conv("k1", 4)
    emit_cepi("k1", 3)
    emit_conv("k1", 5)
    emit_cepi("k1", 4)
    for b in range(12, 16):
        emit_v(b)
        if b % 2:
            emit_vepi(b - 1)
    emit_vt(2)
    emit_conv("q1", 0)
    emit_cepi("k1", 5)
    emit_cepi("q1", 0)
    for j in range(1, 4):
        emit_conv("q1", j)
        emit_cepi("q1", j)
    emit_vt(3)
    emit_conv("k2", 4)
    emit_cepi("k2", 4)
    emit_conv("k2", 5)
    emit_cepi("k2", 5)
    emit_conv("k2", 0)
    emit_cepi("k2", 0)
    emit_conv("k2", 1)
    emit_cepi("k2", 1)
    emit_conv("k2", 2)
    emit_cepi("k2", 2)
    emit_conv("k2", 3)
    emit_cepi("k2", 3)
    emit_conv("q2", 0)
    emit_cepi("q2", 0)
    emit_conv("q2", 1)
    emit_cepi("q2", 1)
    emit_S_quad(0)
    emit_at(0)
    emit_conv("q2", 2)
    emit_cepi("q2", 2)
    emit_conv("q2", 3)
    emit_cepi("q2", 3)
    emit_S_quad(1)
    emit_at(1)
    emit_S_quad(2)
    emit_S_quad(3)
    emit_avpair(0)
    emit_at_pair(8)
    emit_avpair(2)
    emit_at_pair(10)
    emit_avpair(4)
    emit_at_pair(12)
    emit_avpair(6)
    emit_at_pair(14)
    emit_avpair(8)
    emit_avpair(10)
    emit_o(0)
    emit_o(1)
    emit_avpair(12)
    emit_avpair(14)
    emit_o(2)
    emit_o(3)

    if DEBUG_OUTS:
        dbg = {
            "d_q": q.rearrange("p a b -> p (a b)"),
            "d_kpad": kpad.rearrange("p r w -> p (r w)"),
            "d_vn": vn16.rearrange("p a b -> p (a b)"),
            "d_vt": vt16.rearrange("p a b c -> p (a b c)"),
            "d_attn": attn.rearrange("p r w -> p (r w)"),
            "d_z": z16,
            "d_rz": rz16,
            "d_am": am8.rearrange("p a b -> p (a b)"),
            "d_at": at8.rearrange("p a b c -> p (a b c)"),
        }
        for nm, src in dbg.items():
            dd = nc.dram_tensor(nm, list(src.shape),
                                src.dtype, kind="ExternalOutput").ap()
            for s_ in ("sp", "sa", "sv", "sg"):
                wait("sync", (s_, cnt[s_]))
            op("sync", lambda o=dd, i_=src: nc.sync.dma_start(out=o, in_=i_),
               "sdout", 16)

    # ---- tail barrier ----
    for s_ in ("sp", "sa", "sv", "sg", "sdvt", "sdat", "sdout",
               "sdwa", "sdwb", "sdball", "sdo", "sdm"):
        wait("sync", (s_, cnt[s_]))
    for j in range(XCH):
        wait("sync", (f"sdx{j}", cnt[f"sdx{j}"]))

    # ---- emit ----
    def run(eng_name, eng_obj):
        hwm = {}
        for item in plan[eng_name]:
            if item[0] == "w":
                _, s_, v = item
                if hwm.get(s_, 0) >= v:
                    continue
                hwm[s_] = v
                eng_obj.wait_ge(sems[s_], v)
            else:
                _, fn, s_, inc = item
                inst = fn()
                if s_:
                    inst.then_inc(sems[s_], inc)

    with nc.Block() as block:
        @block.sync
        def _(e):
            run("sync", e)

        @block.tensor
        def _(e):
            run("pe", e)

        @block.scalar
        def _(e):
            run("act", e)

        @block.vector
        def _(e):
            run("dve", e)

        @block.gpsimd
        def _(e):
            run("gp", e)

    return nc


_PROGRAM = None


def _host_inputs(x, w_q1, s_q1, b_q1, w_q2, s_q2, b_q2,
                 w_k1, s_k1, b_k1, w_k2, s_k2, b_k2,
                 w_v, s_v, b_v, w_o, s_o, b_o):
    def foldT(w, s):
        return np.ascontiguousarray((s[:, None] * w).T.astype(ml_dtypes.bfloat16))

    wq1t, wq2t = foldT(w_q1, s_q1), foldT(w_q2, s_q2)
    wk1t, wk2t = foldT(w_k1, s_k1), foldT(w_k2, s_k2)
    wvt = foldT(w_v, s_v)
    wo = s_o[:, None] * w_o
    wat = np.ascontiguousarray(wo[:, :C].T.astype(ml_dtypes.bfloat16))
    wxt = np.ascontiguousarray(wo[:, C:].T.astype(ml_dtypes.bfloat16))

    col = lambda b: np.ascontiguousarray(b.astype(np.float32)[:, None])

    valid = np.zeros((BR * BC, NR, NC_), bool)
    for r in range(BR):
        for c in range(BC):
            p = r * BC + c
            valid[p, r:r + 7, c:c + 7] = True

    X = np.asarray(x, np.float32).reshape(C, H, W)
    wallA = np.concatenate([wk1t, wvt, wq1t], axis=1)
    wallB = np.concatenate(
        [wk2t, wq2t, wat, wxt, np.eye(C, dtype=ml_dtypes.bfloat16)], axis=1)
    shared = dict(walla=np.ascontiguousarray(wallA),
                  wallb=np.ascontiguousarray(wallB))

    e16v = np.float32(np.exp(EXP_SHIFT))
    var_rep = {rc * 3 + cc: (rc, {0: 0, 1: 3, 2: 7}[cc])
               for rc in range(2) for cc in range(3)}

    in_maps = []
    for core in range(NCORES):
        h0 = core * RPC
        xsb = np.zeros((C, HALO, WP), np.float32)
        lo, hi = h0 - PAD, h0 + RPC + PAD
        slo, shi = max(lo, 0), min(hi, H)
        xsb[:, slo - lo:shi - lo, PAD:PAD + W] = X[:, slo:shi]

        maskm = np.empty((NVAR, BR * BC, NN), np.float32)
        for v, (brr, cb) in var_rep.items():
            rowok = np.array([0 <= h0 + brr * BR + ri - PAD < H
                              for ri in range(NR)])
            colok = np.array([0 <= cb * BC + ci - PAD < W
                              for ci in range(NC_)])
            inimg = rowok[:, None] & colok[None, :]
            mb = np.where(valid & inimg[None, :, :], 0.0, MASKV)
            maskm[v] = mb.reshape(BR * BC, NN)
        oobc = np.empty((16, BR * BC), np.float32)
        for b in range(16):
            brr, cb = b // 8, b % 8
            rowok = np.array([0 <= h0 + brr * BR + ri - PAD < H
                              for ri in range(NR)])
            colok = np.array([0 <= cb * BC + ci - PAD < W
                              for ci in range(NC_)])
            inimg = rowok[:, None] & colok[None, :]
            n_oob = (valid & ~inimg[None, :, :]).sum(axis=(1, 2))
            oobc[b] = n_oob * e16v
        m = dict(shared)
        m["xs"] = np.ascontiguousarray(
            xsb.reshape(C, NXP).astype(ml_dtypes.bfloat16))
        m["smask"] = np.ascontiguousarray(
            maskm.transpose(1, 0, 2).reshape(BR * BC, NVAR * NN)
            .astype(ml_dtypes.bfloat16))
        m["oobc"] = np.ascontiguousarray(oobc.T.astype(np.float32))
        m["ball"] = np.ascontiguousarray(np.concatenate(
            [col(b_q1), col(b_q2), col(b_k1), col(b_k2), col(b_v),
             col(b_o), np.zeros((C, 2), np.float32)], axis=1))
        in_maps.append(m)
    return in_maps


def kernel(**inputs):
    global _PROGRAM
    if _PROGRAM is None:
        _PROGRAM = _build_program()
    in_maps = _host_inputs(**{k: np.asarray(v) for k, v in inputs.items()})
    res = run_bass_kernel_spmd(_PROGRAM, in_maps, core_ids=list(range(NCORES)))
    stripes = [np.asarray(r["y"]).astype(np.float32)
               .reshape(C, 4, RPC, 32).transpose(0, 2, 1, 3).reshape(C, RPC, W)
               for r in res.results]
    return np.concatenate(stripes, axis=1).reshape(1, C, H, W)


if __name__ == "__main__":
    rng = np.random.default_rng(0)
    fake = {"x": rng.standard_normal((1, C, H, W)).astype(np.float32)}
    for n in ("q1", "q2", "k1", "k2", "v", "o"):
        cin = 2 * C if n == "o" else C
        fake["w_" + n] = (rng.standard_normal((C, cin)) / np.sqrt(cin)).astype(np.float32)
        fake["s_" + n] = rng.uniform(0.5, 1.5, C).astype(np.float32)
        fake["b_" + n] = (rng.standard_normal(C) * 0.1).astype(np.float32)
    out = kernel(**fake)
    print("kernel output", out.shape, out.dtype)
